# revision 34
# baseline (speedup 1.0000x reference)
"""Causal local self-attention (RoPE, window=512) on 8 Trainium2 NeuronCores.

Sharding: (batch, query-quarter) across 8 cores -> core c handles
b = c // 4, queries [512*(c%4), 512*(c%4)+512).  Each core computes
q/k/v projections for its key window [qe-1024, qe) (zero-padded at the
left edge), RoPE, banded softmax, attention, and its output-row slice
of the final projection.  No collectives needed.

All matmul operands are bf16 (f32 PSUM accumulation): halves HBM
traffic and SBUF footprint and enables the DVE 2x_1p mode on the
elementwise ops (all-bf16 packed operands).

Layouts (all chosen so no on-device transposes are ever needed):
  - Q^T, K^T [head_dim-on-partitions, time]   (projection: lhsT=W^T tile)
  - V natural [time-on-partitions, head_dim]  (projection: lhsT=x^T tile)
  - scores^T [key-on-partitions, query]       (AV: lhsT=[V|1], free denominator)
  - out natural [time, channels]              (projection: lhsT=Y^T tile)

RoPE: rotate_half is a partition-pair swap done with stream_shuffle;
the sign lives in the host-built sin table.  The PSUM projection tile
is evicted once to bf16 SBUF by the (otherwise idle) ACT engine, then
shuffle + two mults + add run on DVE at 2x bf16 rate.

Attention is band-exact at 128-query granularity: within each
256-query block, edge key-subtiles s0/s5 are only valid for half the
queries, so their score/AV matmuls run 128-wide and AV accumulates per
128-wide query region (clean PSUM start/stop groups).

Engine balance per core (sim): PE ~118us busy / 137us span (scores
paired per-head in disjoint PE row groups), DVE ~80us, ACT ~78us.
K/Q projection for the first two o-tiles is issued before the V
projection so rope/attention work overlaps the V matmul burst from the
start; weights load once in rep 0 and stay SBUF-resident.
"""

import sys
sys.path.insert(0, '/opt/trn_rl_repo')

from contextlib import ExitStack

import numpy as np
import ml_dtypes

from concourse import bass, bacc, mybir, tile
from concourse.bass_utils import run_bass_kernel_spmd

F32 = mybir.dt.float32
BF16 = mybir.dt.bfloat16
NP_BF16 = ml_dtypes.bfloat16

B, T, C, H, HD = 2, 2048, 1024, 16, 64
LOCAL_WINDOW = 512
ROPE_BASE = 10000.0
NCORES = 8
QL = 512     # queries per core
KL = 1024    # key window per core
QB = 256     # query block (free dim of transposed scores)
NSUB = 6     # key subtiles (of 128) per query block
SCALE = 1.0 / 8.0  # 1/sqrt(HD), folded into Wq on the host

SWAP_MASK = [i ^ 1 for i in range(32)]

# packed pes layout per (qbi, head): key-subtile query-spans within the
# 256-query block (s0 valid only for queries [0:128), s5 only [128:256))
SUB_W = [128, 256, 256, 256, 256, 128]    # query-span width per subtile
SUB_Q0 = [0, 0, 0, 0, 0, 128]             # query-span start per subtile
PES_OFF = [0, 128, 384, 640, 896, 1152]   # column offset in pes


def build_nc(reps=1):
    nc = bacc.Bacc("TRN2", target_bir_lowering=False, debug=False,
                   num_devices=NCORES)

    xT_d = nc.dram_tensor("xT", [C, KL], BF16, kind="ExternalInput").ap()
    wT_d = nc.dram_tensor("wT", [C, 3 * C], BF16, kind="ExternalInput").ap()
    wpT_d = nc.dram_tensor("wpT", [C, C], BF16, kind="ExternalInput").ap()
    biasb_d = nc.dram_tensor("biasb", [128, C], F32, kind="ExternalInput").ap()
    cosq_d = nc.dram_tensor("cosq", [128, QL], BF16, kind="ExternalInput").ap()
    sinq_d = nc.dram_tensor("sinq", [128, QL], BF16, kind="ExternalInput").ap()
    cosk_d = nc.dram_tensor("cosk", [128, KL], BF16, kind="ExternalInput").ap()
    sink_d = nc.dram_tensor("sink", [128, KL], BF16, kind="ExternalInput").ap()
    mask_d = nc.dram_tensor("mask", [128, 2 * 768], BF16,
                            kind="ExternalInput").ap()
    vone_d = nc.dram_tensor("vone", [128, 8 * H], BF16,
                            kind="ExternalInput").ap()
    out_d = nc.dram_tensor("out", [QL, C], F32, kind="ExternalOutput").ap()

    with tile.TileContext(nc) as tc, ExitStack() as top:
        const = top.enter_context(tc.tile_pool(name="const", bufs=1))
        persist = top.enter_context(tc.tile_pool(name="persist", bufs=1))

        cosq_sb = const.tile([128, QL], BF16, tag="cosq", name="cosq")
        sinq_sb = const.tile([128, QL], BF16, tag="sinq", name="sinq")
        cosk_sb = const.tile([128, KL], BF16, tag="cosk", name="cosk")
        sink_sb = const.tile([128, KL], BF16, tag="sink", name="sink")
        mask_sb = const.tile([128, 2 * 768], BF16, tag="mask", name="mask")
        bias_sb = const.tile([128, C], F32, tag="bias", name="bias")
        # const tables go on the gpsimd software-DGE queue so they don't
        # occupy the sync-engine queue that feeds the critical per-rep
        # xT loads
        nc.gpsimd.dma_start(out=cosq_sb, in_=cosq_d)
        nc.gpsimd.dma_start(out=sinq_sb, in_=sinq_d)
        nc.gpsimd.dma_start(out=cosk_sb, in_=cosk_d)
        nc.gpsimd.dma_start(out=sink_sb, in_=sink_d)
        nc.gpsimd.dma_start(out=mask_sb, in_=mask_d)
        nc.gpsimd.dma_start(out=bias_sb, in_=biasb_d)

        qT = [persist.tile([128, QL], BF16, tag=f"qT{i}", name=f"qT{i}")
              for i in range(8)]
        yT = qT  # reuse: yT[ot] slices are written only after the
        #  corresponding qT[ot] slices' last reader
        kT = [persist.tile([128, KL], BF16, tag=f"kT{i}", name=f"kT{i}")
              for i in range(8)]
        v1 = [persist.tile([128, H * (HD + 1)], BF16, tag=f"v1{i}",
                           name=f"v1{i}")
              for i in range(8)]

        def rope_evict(dest_slice, psm, rpool, cosT, sinT):
            """dest = psm*cos + swap_pairs(psm)*sin_signed.

            ACT evicts PSUM once to bf16; shuffle + mults + add on DVE
            run at 2x (all-bf16 packed operands)."""
            c = rpool.tile([128, 512], BF16, tag="c", name="c")
            nc.scalar.copy(c, psm)
            r = rpool.tile([128, 512], BF16, tag="r", name="r")
            nc.vector.stream_shuffle(r, c, SWAP_MASK)
            t1 = rpool.tile([128, 512], BF16, tag="t1", name="t1")
            nc.vector.tensor_tensor(out=t1, in0=c, in1=cosT,
                                    op=mybir.AluOpType.mult)
            t2 = rpool.tile([128, 512], BF16, tag="t2", name="t2")
            nc.vector.tensor_tensor(out=t2, in0=r, in1=sinT,
                                    op=mybir.AluOpType.mult)
            nc.vector.tensor_tensor(out=dest_slice, in0=t1, in1=t2,
                                    op=mybir.AluOpType.add)

        # SBUF pools persist across reps so the next rep's input DMA can
        # overlap this rep's attention/out-proj tail (PSUM pools stay
        # phase-scoped: the 8 banks are fully subscribed per phase).
        xpool = top.enter_context(tc.tile_pool(name="xp", bufs=1))
        wqk = top.enter_context(tc.tile_pool(name="wqk", bufs=1))
        rpool = top.enter_context(tc.tile_pool(name="rp", bufs=2))
        wvpool = top.enter_context(tc.tile_pool(name="wv", bufs=1))
        apool = top.enter_context(tc.tile_pool(name="att", bufs=2))
        dpool = top.enter_context(tc.tile_pool(name="div", bufs=2))
        wopool = top.enter_context(tc.tile_pool(name="wo", bufs=1))
        opool = top.enter_context(tc.tile_pool(name="ob", bufs=2))

        # Weights are loaded once (during rep 0, interleaved with the
        # xT loads in consumption order on the sync queue) and stay
        # SBUF-resident across reps.
        wkq_all = [wqk.tile([128, 2 * C], BF16, tag=f"wkq{i}",
                            name=f"wkq{i}")
                   for i in range(8)]
        wv = [wvpool.tile([128, C], BF16, tag=f"wv{i}", name=f"wv{i}")
              for i in range(8)]
        wp = [wopool.tile([128, C], BF16, tag=f"wo{i}", name=f"wo{i}")
              for i in range(8)]
        # v1 "ones" columns are written once (compute only overwrites
        # the value columns); gpsimd queue, off the critical path
        for tt in range(8):
            nc.gpsimd.dma_start(
                out=v1[tt].rearrange("p (h x) -> p h x",
                                     x=HD + 1)[:, :, HD:HD + 1],
                in_=vone_d[:, tt * H:(tt + 1) * H])

        for _rep in range(reps):
            with ExitStack() as ph:
                psm_p = ph.enter_context(
                    tc.tile_pool(name="psm", bufs=2, space="PSUM"))

                xT_sb = [xpool.tile([128, KL], BF16, tag=f"xT{i}",
                                    name=f"xT{i}")
                         for i in range(8)]
                for ct in range(8):
                    nc.sync.dma_start(
                        out=xT_sb[ct][:, 0:512],
                        in_=xT_d[ct * 128:(ct + 1) * 128, 0:512])
                if _rep == 0:
                    for ct in range(8):
                        nc.sync.dma_start(
                            out=wkq_all[ct][:, 0:256],
                            in_=wT_d[ct * 128:(ct + 1) * 128, 0:256])
                for ct in range(8):
                    nc.sync.dma_start(
                        out=xT_sb[ct][:, 512:1024],
                        in_=xT_d[ct * 128:(ct + 1) * 128, 512:1024])
                if _rep == 0:
                    for ct in range(8):
                        nc.sync.dma_start(
                            out=wkq_all[ct][:, 256:512],
                            in_=wT_d[ct * 128:(ct + 1) * 128, 256:512])
                    for ct in range(8):
                        nc.sync.dma_start(
                            out=wv[ct][:, 0:512],
                            in_=wT_d[ct * 128:(ct + 1) * 128,
                                     2 * C:2 * C + 512])
                    for ct in range(8):
                        nc.sync.dma_start(
                            out=wv[ct][:, 512:1024],
                            in_=wT_d[ct * 128:(ct + 1) * 128,
                                     2 * C + 512:3 * C])
                    for ct in range(8):
                        nc.sync.dma_start(
                            out=wkq_all[ct][:, 512:2048],
                            in_=wT_d[ct * 128:(ct + 1) * 128, 512:2048])
                    for ct in range(8):
                        nc.sync.dma_start(
                            out=wp[ct],
                            in_=wpT_d[ct * 128:(ct + 1) * 128, :])

                def kq_project(ot):
                    """K (two 512-wide halves) + Q projection and rope
                    for o-tile ot."""
                    wk = [wkq_all[ct][:, ot * 256:ot * 256 + 128]
                          for ct in range(8)]
                    wq = [wkq_all[ct][:, ot * 256 + 128:ot * 256 + 256]
                          for ct in range(8)]
                    for th in range(2):
                        psm = psm_p.tile([128, 512], F32, tag="psm",
                                         name="psm")
                        for ct in range(8):
                            nc.tensor.matmul(
                                psm, wk[ct],
                                xT_sb[ct][:, th * 512:(th + 1) * 512],
                                start=(ct == 0), stop=(ct == 7))
                        rope_evict(kT[ot][:, th * 512:(th + 1) * 512],
                                   psm, rpool,
                                   cosk_sb[:, th * 512:(th + 1) * 512],
                                   sink_sb[:, th * 512:(th + 1) * 512])
                    psm = psm_p.tile([128, 512], F32, tag="psm", name="psm")
                    for ct in range(8):
                        nc.tensor.matmul(
                            psm, wq[ct], xT_sb[ct][:, 512:1024],
                            start=(ct == 0), stop=(ct == 7))
                    rope_evict(qT[ot], psm, rpool, cosq_sb, sinq_sb)

                # K/Q for the first two o-tiles: gives DVE/ACT rope work
                # to overlap with the V projection burst below.
                kq_project(0)
                kq_project(1)

                # ---- V projection ----
                with tc.tile_pool(name="psV", bufs=1, space="PSUM") as psV:
                    # two groups of 4 time-tiles: 4 pv banks + 2 psm
                    # banks (kq_project) stay within the 8 PSUM banks
                    for oh in range(2):
                        for g in range(2):
                            pv = [psV.tile([128, 512], F32, tag=f"pv{i}",
                                           name=f"pv{i}")
                                  for i in range(4)]
                            for ct in range(8):
                                for ti in range(4):
                                    tt = g * 4 + ti
                                    lhs = xT_sb[ct][
                                        :, tt * 128:(tt + 1) * 128]
                                    nc.tensor.matmul(
                                        pv[ti], lhs,
                                        wv[ct][:, oh * 512:(oh + 1) * 512],
                                        start=(ct == 0), stop=(ct == 7))
                            for ti in range(4):
                                tt = g * 4 + ti
                                dst = v1[tt].rearrange(
                                    "p (h x) -> p h x", x=HD + 1)[
                                    :, oh * 8:(oh + 1) * 8, 0:HD]
                                src = pv[ti].rearrange(
                                    "p (h x) -> p h x", x=HD)
                                # NB: must stay on ACT -- DVE tensor_copy
                                # into a strided tile miscomputed on HW
                                # before.
                                nc.scalar.copy(dst, src)

                # ---- attention per o-tile (head pair 2*ot, 2*ot+1) ----
                with ExitStack() as ph2:
                    aps = ph2.enter_context(
                        tc.tile_pool(name="attps", bufs=2, space="PSUM"))
                    ypool = ph2.enter_context(
                        tc.tile_pool(name="yps", bufs=1, space="PSUM"))
                    for ot in range(8):
                        # paired score matmuls for heads 2*ot / 2*ot+1
                        # land in disjoint PE row-groups (partitions
                        # 0-63 / 64-127) and execute concurrently.
                        yps = [ypool.tile([65, 2 * QB], F32, tag=f"yp{hh}",
                                          name=f"yp{hh}")
                               for hh in range(2)]
                        for qbi in range(2):
                            qb = qbi * QB
                            # band-exact edge subtiles: s0 only covers
                            # queries [0:128) of the block and s5 only
                            # [128:256), so their score/AV matmuls run at
                            # 128-wide free dim.  Packed pes layout:
                            # [s0:128|s1:256|s2:256|s3:256|s4:256|s5:128].
                            pes = [apool.tile([128, 1280], BF16,
                                              tag=f"pe{hh}", name=f"pe{hh}")
                                   for hh in range(2)]
                            for sa, sb in ((0, 1), (2, 3), (4, 5)):
                                pp = [aps.tile([128, 2 * QB], F32,
                                               tag=f"ps{hh}", name=f"ps{hh}")
                                      for hh in range(2)]
                                off = 0
                                for s in (sa, sb):
                                    w = SUB_W[s]
                                    q0 = qb + SUB_Q0[s]
                                    k0 = qb + s * 128
                                    for hh in range(2):
                                        po = hh * 64
                                        nc.tensor.matmul(
                                            pp[hh][:, off:off + w],
                                            kT[ot][po:po + 64, k0:k0 + 128],
                                            qT[ot][po:po + 64, q0:q0 + w],
                                            start=True, stop=True)
                                    off += w
                                for hh in range(2):
                                    nc.scalar.activation(
                                        pes[hh][:, PES_OFF[sa]:
                                                PES_OFF[sa] + off],
                                        pp[hh][:, 0:off],
                                        mybir.ActivationFunctionType.Exp)
                            mb0 = qbi * 768
                            for hh in range(2):
                                pe = pes[hh]
                                nc.vector.tensor_tensor(
                                    out=pe[:, 0:384], in0=pe[:, 0:384],
                                    in1=mask_sb[:, mb0:mb0 + 384],
                                    op=mybir.AluOpType.mult)
                                nc.vector.tensor_tensor(
                                    out=pe[:, 896:1280],
                                    in0=pe[:, 896:1280],
                                    in1=mask_sb[:, mb0 + 384:mb0 + 768],
                                    op=mybir.AluOpType.mult)
                            # AV per 128-wide query region (clean PSUM
                            # accumulation groups: region 0 sums subtiles
                            # 0-4, region 1 sums 1-5)
                            for reg in range(2):
                                subs = (0, 1, 2, 3, 4) if reg == 0 \
                                    else (1, 2, 3, 4, 5)
                                yd = qb + reg * 128
                                for i, s in enumerate(subs):
                                    col = PES_OFF[s] + reg * 128 - SUB_Q0[s]
                                    for hh in range(2):
                                        h = 2 * ot + hh
                                        vt = v1[qbi * 2 + s][
                                            :, (HD + 1) * h:
                                            (HD + 1) * h + HD + 1]
                                        nc.tensor.matmul(
                                            yps[hh][:, yd:yd + 128], vt,
                                            pes[hh][:, col:col + 128],
                                            start=(i == 0), stop=(i == 4))
                        for hh in range(2):
                            po = hh * 64
                            rd = dpool.tile([1, 2 * QB], F32, tag="rd",
                                            name="rd")
                            nc.vector.reciprocal(rd, yps[hh][64:65, :])
                            rb = dpool.tile([64, 2 * QB], F32, tag="rb",
                                            name="rb")
                            nc.gpsimd.partition_broadcast(rb, rd)
                            nc.vector.tensor_tensor(
                                out=yT[ot][po:po + 64, :],
                                in0=yps[hh][0:64, :], in1=rb,
                                op=mybir.AluOpType.mult)
                        # interleave the next-but-one K/Q projection so
                        # PE alternates proj/attention and rope keeps
                        # DVE fed.
                        if ot + 2 < 8:
                            kq_project(ot + 2)

            # ---------------- output projection ----------------------
            with ExitStack() as phd:
                psO = phd.enter_context(
                    tc.tile_pool(name="psO", bufs=2, space="PSUM"))
                for tt in range(4):
                    po_ = psO.tile([128, C], F32, tag="psO", name="psO")
                    for ct in range(8):
                        lhs = yT[ct][:, tt * 128:(tt + 1) * 128]
                        st, sp = (ct == 0), (ct == 7)
                        for hh in range(2):
                            sl = slice(hh * 512, (hh + 1) * 512)
                            nc.tensor.matmul(
                                po_[:, sl], lhs, wp[ct][:, sl],
                                start=st, stop=sp)
                    ob = opool.tile([128, C], F32, tag="ob", name="ob")
                    nc.vector.tensor_tensor(
                        out=ob, in0=po_, in1=bias_sb,
                        op=mybir.AluOpType.add)
                    nc.sync.dma_start(
                        out=out_d[tt * 128:(tt + 1) * 128, :], in_=ob)

    nc.compile()
    return nc


# ---------------------------------------------------------------------
# host side
# ---------------------------------------------------------------------

def _trig_tables(positions, n):
    """cos / signed-sin tables in transposed layout [128, n].

    Row p corresponds to head-dim d = p % 64; freq index d//2.  The sin
    table carries the rotate_half sign: -sin on even d, +sin on odd d,
    so that q' = q*cos + swap_pairs(q)*sin_signed.
    """
    inv = 1.0 / (ROPE_BASE ** (np.arange(HD // 2, dtype=np.float32)
                               / (HD // 2)))
    freqs = positions.astype(np.float32)[None, :] * inv[:, None]  # [32, n]
    cos = np.repeat(np.cos(freqs), 2, axis=0)  # [64, n]
    sin = np.repeat(np.sin(freqs), 2, axis=0)
    sign = np.where(np.arange(64) % 2 == 0, -1.0, 1.0).astype(np.float32)
    sin = sin * sign[:, None]
    return (np.ascontiguousarray(np.tile(cos, (2, 1))).astype(NP_BF16),
            np.ascontiguousarray(np.tile(sin, (2, 1))).astype(NP_BF16))


def _masks(qs):
    """0/1 mask tiles [128, 2*768]: scores^T layout [key-part, query-free].

    Packed per qbi block: [s0(128) s1(256) s4(256) s5(128)] matching the
    band-exact pes layout; subtiles 2,3 are fully in-band (no mask)."""
    p = np.arange(128)[:, None]
    m = np.zeros((128, 2, 768), np.float32)
    spec = [(0, 0, 128, 0), (1, 128, 256, 0),
            (4, 384, 256, 0), (5, 640, 128, 128)]
    for qi, qb in enumerate((0, QB)):
        for s, col, w, x0 in spec:
            x = x0 + np.arange(w)[None, :]
            pk = (qs - LOCAL_WINDOW) + qb + 128 * s + p  # global key pos
            band = (x >= 128 * s + p - LOCAL_WINDOW) & (x <= 128 * s + p)
            m[:, qi, col:col + w] = (band & (pk >= 0)).astype(np.float32)
    return np.ascontiguousarray(m.reshape(128, 2 * 768)).astype(NP_BF16)


def _host_inputs(x, Wqkv, Wproj, bproj):
    # 1/sqrt(hd) folded into the (linear) q projection
    Wq, Wk, Wv = Wqkv[0:C] * SCALE, Wqkv[C:2 * C], Wqkv[2 * C:3 * C]
    # slab layout: per o-tile [K(128) | Q(128)] interleaved, then V
    WqT, WkT, WvT = Wq.T, Wk.T, Wv.T
    slab = np.empty((C, 2 * C), np.float32)
    for ot in range(8):
        slab[:, ot * 256:ot * 256 + 128] = WkT[:, ot * 128:(ot + 1) * 128]
        slab[:, ot * 256 + 128:ot * 256 + 256] = \
            WqT[:, ot * 128:(ot + 1) * 128]
    wT = np.ascontiguousarray(
        np.concatenate([slab, WvT], axis=1)).astype(NP_BF16)
    wpT = np.ascontiguousarray(Wproj.T).astype(NP_BF16)
    biasb = np.ascontiguousarray(
        np.broadcast_to(bproj, (128, C))).astype(np.float32)

    in_maps = []
    for core in range(NCORES):
        b, qi = divmod(core, 4)
        qs = qi * QL
        qe = qs + QL
        lo = qe - KL
        xw = np.zeros((KL, C), np.float32)
        src_lo = max(lo, 0)
        xw[src_lo - lo:, :] = x[b, src_lo:qe, :]
        xT = np.ascontiguousarray(xw.T).astype(NP_BF16)

        cosq, sinq = _trig_tables(qs + np.arange(QL), QL)
        cosk, sink = _trig_tables(lo + np.arange(KL), KL)
        # validity column for [V|1]: 1.0 where the key row is a real
        # (non-padding) position, per v-time-tile, repeated per head
        jpos = lo + np.arange(KL)
        v8 = (jpos >= 0).astype(np.float32).reshape(8, 128)  # [tt, p]
        vone = np.ascontiguousarray(
            np.repeat(v8[:, :, None], H, axis=2)             # [tt, p, h]
            .transpose(1, 0, 2).reshape(128, 8 * H)).astype(NP_BF16)
        in_maps.append({
            "xT": xT, "wT": wT, "wpT": wpT, "biasb": biasb,
            "cosq": cosq, "sinq": sinq, "cosk": cosk, "sink": sink,
            "mask": _masks(qs), "vone": vone,
        })
    return in_maps


_NC_CACHE = {}


def _get_nc(reps=1):
    key = reps
    if key not in _NC_CACHE:
        _NC_CACHE[key] = build_nc(reps=reps)
    return _NC_CACHE[key]


def kernel(x, Wqkv, Wproj, bproj):
    x = np.asarray(x, dtype=np.float32)
    Wqkv = np.asarray(Wqkv, dtype=np.float32)
    Wproj = np.asarray(Wproj, dtype=np.float32)
    bproj = np.asarray(bproj, dtype=np.float32)
    nc = _get_nc()
    in_maps = _host_inputs(x, Wqkv, Wproj, bproj)
    res = run_bass_kernel_spmd(nc, in_maps, list(range(NCORES)))
    out = np.empty((B, T, C), np.float32)
    for core in range(NCORES):
        b, qi = divmod(core, 4)
        out[b, qi * QL:(qi + 1) * QL, :] = res.results[core]["out"]
    return out


# revision 38
# speedup vs baseline: 2.2333x; 2.2333x over previous
"""Causal local self-attention (RoPE, window=512) on 8 Trainium2 NeuronCores.

Sharding: (batch, query-quarter) across 8 cores -> core c handles
b = c // 4, queries [512*(c%4), 512*(c%4)+512).  Each core computes
q/k/v projections for its key window [qe-1024, qe) (zero-padded at the
left edge), RoPE, banded softmax, attention, and its output-row slice
of the final projection.  No collectives needed.

All matmul operands are bf16 (f32 PSUM accumulation): halves HBM
traffic and SBUF footprint and enables the DVE 2x_1p mode on the
elementwise ops (all-bf16 packed operands).

Layouts (all chosen so no on-device transposes are ever needed):
  - Q^T, K^T [head_dim-on-partitions, time]   (projection: lhsT=W^T tile)
  - V natural [time-on-partitions, head_dim]  (projection: lhsT=x^T tile)
  - scores^T [key-on-partitions, query]       (AV: lhsT=[V|1], free denominator)
  - out natural [time, channels]              (projection: lhsT=Y^T tile)

RoPE: rotate_half is a partition-pair swap done with stream_shuffle;
the sign lives in the host-built sin table.  The PSUM projection tile
is evicted once to bf16 SBUF by the (otherwise idle) ACT engine, then
shuffle + two mults + add run on DVE at 2x bf16 rate.

Attention is band-exact at 128-query granularity: within each
256-query block, edge key-subtiles s0/s5 are only valid for half the
queries, so their score/AV matmuls run 128-wide and AV accumulates per
128-wide query region (clean PSUM start/stop groups).

Engine balance per core (sim): PE ~118us busy / 137us span (scores
paired per-head in disjoint PE row groups), DVE ~80us, ACT ~78us.
K/Q projection for the first two o-tiles is issued before the V
projection so rope/attention work overlaps the V matmul burst from the
start; weights load once in rep 0 and stay SBUF-resident.
"""

import sys
sys.path.insert(0, '/opt/trn_rl_repo')

from contextlib import ExitStack

import numpy as np
import ml_dtypes

from concourse import bass, bacc, mybir, tile
from concourse.bass_utils import run_bass_kernel_spmd

F32 = mybir.dt.float32
BF16 = mybir.dt.bfloat16
NP_BF16 = ml_dtypes.bfloat16

B, T, C, H, HD = 2, 2048, 1024, 16, 64
LOCAL_WINDOW = 512
ROPE_BASE = 10000.0
NCORES = 8
QL = 512     # queries per core
KL = 1024    # key window per core
QB = 256     # query block (free dim of transposed scores)
NSUB = 6     # key subtiles (of 128) per query block
SCALE = 1.0 / 8.0  # 1/sqrt(HD), folded into Wq on the host

SWAP_MASK = [i ^ 1 for i in range(32)]

# packed pes layout per (qbi, head): key-subtile query-spans within the
# 256-query block (s0 valid only for queries [0:128), s5 only [128:256))
SUB_W = [128, 256, 256, 256, 256, 128]    # query-span width per subtile
SUB_Q0 = [0, 0, 0, 0, 0, 128]             # query-span start per subtile
PES_OFF = [0, 128, 384, 640, 896, 1152]   # column offset in pes


def build_nc(reps=1):
    nc = bacc.Bacc("TRN2", target_bir_lowering=False, debug=False,
                   num_devices=NCORES)

    xT_d = nc.dram_tensor("xT", [C, KL], BF16, kind="ExternalInput").ap()
    wT_d = nc.dram_tensor("wT", [C, 3 * C], BF16, kind="ExternalInput").ap()
    wpT_d = nc.dram_tensor("wpT", [C, C], BF16, kind="ExternalInput").ap()
    biasb_d = nc.dram_tensor("biasb", [128, C], F32, kind="ExternalInput").ap()
    cosq_d = nc.dram_tensor("cosq", [128, QL], BF16, kind="ExternalInput").ap()
    sinq_d = nc.dram_tensor("sinq", [128, QL], BF16, kind="ExternalInput").ap()
    cosk_d = nc.dram_tensor("cosk", [128, KL], BF16, kind="ExternalInput").ap()
    sink_d = nc.dram_tensor("sink", [128, KL], BF16, kind="ExternalInput").ap()
    mask_d = nc.dram_tensor("mask", [128, 2 * 768], BF16,
                            kind="ExternalInput").ap()
    vone_d = nc.dram_tensor("vone", [128, 8 * H], BF16,
                            kind="ExternalInput").ap()
    out_d = nc.dram_tensor("out", [QL, C], F32, kind="ExternalOutput").ap()

    with tile.TileContext(nc) as tc, ExitStack() as top:
        const = top.enter_context(tc.tile_pool(name="const", bufs=1))
        persist = top.enter_context(tc.tile_pool(name="persist", bufs=1))

        cosq_sb = const.tile([128, QL], BF16, tag="cosq", name="cosq")
        sinq_sb = const.tile([128, QL], BF16, tag="sinq", name="sinq")
        cosk_sb = const.tile([128, KL], BF16, tag="cosk", name="cosk")
        sink_sb = const.tile([128, KL], BF16, tag="sink", name="sink")
        mask_sb = const.tile([128, 2 * 768], BF16, tag="mask", name="mask")
        bias_sb = const.tile([128, C], F32, tag="bias", name="bias")
        # const tables go on the gpsimd software-DGE queue so they don't
        # occupy the sync-engine queue that feeds the critical per-rep
        # xT loads
        nc.gpsimd.dma_start(out=cosq_sb, in_=cosq_d)
        nc.gpsimd.dma_start(out=sinq_sb, in_=sinq_d)
        nc.gpsimd.dma_start(out=cosk_sb, in_=cosk_d)
        nc.gpsimd.dma_start(out=sink_sb, in_=sink_d)
        nc.gpsimd.dma_start(out=mask_sb, in_=mask_d)
        nc.gpsimd.dma_start(out=bias_sb, in_=biasb_d)

        qT = [persist.tile([128, QL], BF16, tag=f"qT{i}", name=f"qT{i}")
              for i in range(8)]
        yT = qT  # reuse: yT[ot] slices are written only after the
        #  corresponding qT[ot] slices' last reader
        kT = [persist.tile([128, KL], BF16, tag=f"kT{i}", name=f"kT{i}")
              for i in range(8)]
        v1 = [persist.tile([128, H * (HD + 1)], BF16, tag=f"v1{i}",
                           name=f"v1{i}")
              for i in range(8)]

        def rope_evict(dest_slice, psm, rpool, cosT, sinT):
            """dest = psm*cos + swap_pairs(psm)*sin_signed.

            ACT evicts PSUM once to bf16; shuffle + mults + add on DVE
            run at 2x (all-bf16 packed operands)."""
            c = rpool.tile([128, 512], BF16, tag="c", name="c")
            nc.scalar.copy(c, psm)
            r = rpool.tile([128, 512], BF16, tag="r", name="r")
            nc.vector.stream_shuffle(r, c, SWAP_MASK)
            t1 = rpool.tile([128, 512], BF16, tag="t1", name="t1")
            nc.vector.tensor_tensor(out=t1, in0=c, in1=cosT,
                                    op=mybir.AluOpType.mult)
            t2 = rpool.tile([128, 512], BF16, tag="t2", name="t2")
            nc.vector.tensor_tensor(out=t2, in0=r, in1=sinT,
                                    op=mybir.AluOpType.mult)
            nc.vector.tensor_tensor(out=dest_slice, in0=t1, in1=t2,
                                    op=mybir.AluOpType.add)

        # SBUF pools persist across reps so the next rep's input DMA can
        # overlap this rep's attention/out-proj tail (PSUM pools stay
        # phase-scoped: the 8 banks are fully subscribed per phase).
        xpool = top.enter_context(tc.tile_pool(name="xp", bufs=1))
        wqk = top.enter_context(tc.tile_pool(name="wqk", bufs=1))
        rpool = top.enter_context(tc.tile_pool(name="rp", bufs=2))
        # the projection PSUM pool (2 banks) persists too: every phase
        # fits alongside it (V: +4, attention: +6, out-proj: +4), and a
        # per-rep pool close/open would serialize the rep boundary
        psm_p = top.enter_context(
            tc.tile_pool(name="psm", bufs=2, space="PSUM"))
        wvpool = top.enter_context(tc.tile_pool(name="wv", bufs=1))
        apool = top.enter_context(tc.tile_pool(name="att", bufs=2))
        dpool = top.enter_context(tc.tile_pool(name="div", bufs=2))
        wopool = top.enter_context(tc.tile_pool(name="wo", bufs=1))
        opool = top.enter_context(tc.tile_pool(name="ob", bufs=2))

        # Weights are loaded once (during rep 0, interleaved with the
        # xT loads in consumption order on the sync queue) and stay
        # SBUF-resident across reps.
        wkq_all = [wqk.tile([128, 2 * C], BF16, tag=f"wkq{i}",
                            name=f"wkq{i}")
                   for i in range(8)]
        wv = [wvpool.tile([128, C], BF16, tag=f"wv{i}", name=f"wv{i}")
              for i in range(8)]
        wp = [wopool.tile([128, C], BF16, tag=f"wo{i}", name=f"wo{i}")
              for i in range(8)]
        # v1 "ones" columns are written once (compute only overwrites
        # the value columns); gpsimd queue, off the critical path
        for tt in range(8):
            nc.gpsimd.dma_start(
                out=v1[tt].rearrange("p (h x) -> p h x",
                                     x=HD + 1)[:, :, HD:HD + 1],
                in_=vone_d[:, tt * H:(tt + 1) * H])

        for _rep in range(reps):
            with ExitStack() as ph:
                xT_sb = [xpool.tile([128, KL], BF16, tag=f"xT{i}",
                                    name=f"xT{i}")
                         for i in range(8)]
                for ct in range(8):
                    nc.sync.dma_start(
                        out=xT_sb[ct][:, 0:512],
                        in_=xT_d[ct * 128:(ct + 1) * 128, 0:512])
                if _rep == 0:
                    for ct in range(8):
                        nc.sync.dma_start(
                            out=wkq_all[ct][:, 0:256],
                            in_=wT_d[ct * 128:(ct + 1) * 128, 0:256])
                for ct in range(8):
                    nc.sync.dma_start(
                        out=xT_sb[ct][:, 512:1024],
                        in_=xT_d[ct * 128:(ct + 1) * 128, 512:1024])
                if _rep == 0:
                    for ct in range(8):
                        nc.sync.dma_start(
                            out=wkq_all[ct][:, 256:512],
                            in_=wT_d[ct * 128:(ct + 1) * 128, 256:512])
                    for ct in range(8):
                        nc.sync.dma_start(
                            out=wv[ct][:, 0:512],
                            in_=wT_d[ct * 128:(ct + 1) * 128,
                                     2 * C:2 * C + 512])
                    for ct in range(8):
                        nc.sync.dma_start(
                            out=wv[ct][:, 512:1024],
                            in_=wT_d[ct * 128:(ct + 1) * 128,
                                     2 * C + 512:3 * C])
                    for ct in range(8):
                        nc.sync.dma_start(
                            out=wkq_all[ct][:, 512:2048],
                            in_=wT_d[ct * 128:(ct + 1) * 128, 512:2048])
                    for ct in range(8):
                        nc.sync.dma_start(
                            out=wp[ct],
                            in_=wpT_d[ct * 128:(ct + 1) * 128, :])

                def kq_project(ot):
                    """K (two 512-wide halves) + Q projection and rope
                    for o-tile ot."""
                    wk = [wkq_all[ct][:, ot * 256:ot * 256 + 128]
                          for ct in range(8)]
                    wq = [wkq_all[ct][:, ot * 256 + 128:ot * 256 + 256]
                          for ct in range(8)]
                    for th in range(2):
                        psm = psm_p.tile([128, 512], F32, tag="psm",
                                         name="psm")
                        for ct in range(8):
                            nc.tensor.matmul(
                                psm, wk[ct],
                                xT_sb[ct][:, th * 512:(th + 1) * 512],
                                start=(ct == 0), stop=(ct == 7))
                        rope_evict(kT[ot][:, th * 512:(th + 1) * 512],
                                   psm, rpool,
                                   cosk_sb[:, th * 512:(th + 1) * 512],
                                   sink_sb[:, th * 512:(th + 1) * 512])
                    psm = psm_p.tile([128, 512], F32, tag="psm", name="psm")
                    for ct in range(8):
                        nc.tensor.matmul(
                            psm, wq[ct], xT_sb[ct][:, 512:1024],
                            start=(ct == 0), stop=(ct == 7))
                    rope_evict(qT[ot], psm, rpool, cosq_sb, sinq_sb)

                # K/Q for the first two o-tiles: gives DVE/ACT rope work
                # to overlap with the V projection burst below.
                kq_project(0)
                kq_project(1)

                # ---- V projection ----
                with tc.tile_pool(name="psV", bufs=1, space="PSUM") as psV:
                    # two groups of 4 time-tiles: 4 pv banks + 2 psm
                    # banks (kq_project) stay within the 8 PSUM banks
                    for oh in range(2):
                        for g in range(2):
                            pv = [psV.tile([128, 512], F32, tag=f"pv{i}",
                                           name=f"pv{i}")
                                  for i in range(4)]
                            for ct in range(8):
                                for ti in range(4):
                                    tt = g * 4 + ti
                                    lhs = xT_sb[ct][
                                        :, tt * 128:(tt + 1) * 128]
                                    nc.tensor.matmul(
                                        pv[ti], lhs,
                                        wv[ct][:, oh * 512:(oh + 1) * 512],
                                        start=(ct == 0), stop=(ct == 7))
                            for ti in range(4):
                                tt = g * 4 + ti
                                dst = v1[tt].rearrange(
                                    "p (h x) -> p h x", x=HD + 1)[
                                    :, oh * 8:(oh + 1) * 8, 0:HD]
                                src = pv[ti].rearrange(
                                    "p (h x) -> p h x", x=HD)
                                # NB: must stay on ACT -- DVE tensor_copy
                                # into a strided tile miscomputed on HW
                                # before.
                                nc.scalar.copy(dst, src)

                # ---- attention per o-tile (head pair 2*ot, 2*ot+1) ----
                with ExitStack() as ph2:
                    aps = ph2.enter_context(
                        tc.tile_pool(name="attps", bufs=2, space="PSUM"))
                    ypool = ph2.enter_context(
                        tc.tile_pool(name="yps", bufs=1, space="PSUM"))
                    for ot in range(8):
                        # paired score matmuls for heads 2*ot / 2*ot+1
                        # land in disjoint PE row-groups (partitions
                        # 0-63 / 64-127) and execute concurrently.
                        yps = [ypool.tile([65, 2 * QB], F32, tag=f"yp{hh}",
                                          name=f"yp{hh}")
                               for hh in range(2)]
                        for qbi in range(2):
                            qb = qbi * QB
                            # band-exact edge subtiles: s0 only covers
                            # queries [0:128) of the block and s5 only
                            # [128:256), so their score/AV matmuls run at
                            # 128-wide free dim.  Packed pes layout:
                            # [s0:128|s1:256|s2:256|s3:256|s4:256|s5:128].
                            pes = [apool.tile([128, 1280], BF16,
                                              tag=f"pe{hh}", name=f"pe{hh}")
                                   for hh in range(2)]
                            for sa, sb in ((0, 1), (2, 3), (4, 5)):
                                pp = [aps.tile([128, 2 * QB], F32,
                                               tag=f"ps{hh}", name=f"ps{hh}")
                                      for hh in range(2)]
                                off = 0
                                for s in (sa, sb):
                                    w = SUB_W[s]
                                    q0 = qb + SUB_Q0[s]
                                    k0 = qb + s * 128
                                    for hh in range(2):
                                        po = hh * 64
                                        nc.tensor.matmul(
                                            pp[hh][:, off:off + w],
                                            kT[ot][po:po + 64, k0:k0 + 128],
                                            qT[ot][po:po + 64, q0:q0 + w],
                                            start=True, stop=True)
                                    off += w
                                for hh in range(2):
                                    nc.scalar.activation(
                                        pes[hh][:, PES_OFF[sa]:
                                                PES_OFF[sa] + off],
                                        pp[hh][:, 0:off],
                                        mybir.ActivationFunctionType.Exp)
                            mb0 = qbi * 768
                            for hh in range(2):
                                pe = pes[hh]
                                nc.vector.tensor_tensor(
                                    out=pe[:, 0:384], in0=pe[:, 0:384],
                                    in1=mask_sb[:, mb0:mb0 + 384],
                                    op=mybir.AluOpType.mult)
                                nc.vector.tensor_tensor(
                                    out=pe[:, 896:1280],
                                    in0=pe[:, 896:1280],
                                    in1=mask_sb[:, mb0 + 384:mb0 + 768],
                                    op=mybir.AluOpType.mult)
                            # AV per 128-wide query region (clean PSUM
                            # accumulation groups: region 0 sums subtiles
                            # 0-4, region 1 sums 1-5)
                            for reg in range(2):
                                subs = (0, 1, 2, 3, 4) if reg == 0 \
                                    else (1, 2, 3, 4, 5)
                                yd = qb + reg * 128
                                for i, s in enumerate(subs):
                                    col = PES_OFF[s] + reg * 128 - SUB_Q0[s]
                                    for hh in range(2):
                                        h = 2 * ot + hh
                                        vt = v1[qbi * 2 + s][
                                            :, (HD + 1) * h:
                                            (HD + 1) * h + HD + 1]
                                        nc.tensor.matmul(
                                            yps[hh][:, yd:yd + 128], vt,
                                            pes[hh][:, col:col + 128],
                                            start=(i == 0), stop=(i == 4))
                        for hh in range(2):
                            po = hh * 64
                            rd = dpool.tile([1, 2 * QB], F32, tag="rd",
                                            name="rd")
                            nc.vector.reciprocal(rd, yps[hh][64:65, :])
                            rb = dpool.tile([64, 2 * QB], F32, tag="rb",
                                            name="rb")
                            nc.gpsimd.partition_broadcast(rb, rd)
                            nc.vector.tensor_tensor(
                                out=yT[ot][po:po + 64, :],
                                in0=yps[hh][0:64, :], in1=rb,
                                op=mybir.AluOpType.mult)
                        # interleave the next-but-one K/Q projection so
                        # PE alternates proj/attention and rope keeps
                        # DVE fed.
                        if ot + 2 < 8:
                            kq_project(ot + 2)

            # ---------------- output projection ----------------------
            with ExitStack() as phd:
                psO = phd.enter_context(
                    tc.tile_pool(name="psO", bufs=2, space="PSUM"))
                for tt in range(4):
                    po_ = psO.tile([128, C], F32, tag="psO", name="psO")
                    for ct in range(8):
                        lhs = yT[ct][:, tt * 128:(tt + 1) * 128]
                        st, sp = (ct == 0), (ct == 7)
                        for hh in range(2):
                            sl = slice(hh * 512, (hh + 1) * 512)
                            nc.tensor.matmul(
                                po_[:, sl], lhs, wp[ct][:, sl],
                                start=st, stop=sp)
                    ob = opool.tile([128, C], F32, tag="ob", name="ob")
                    nc.vector.tensor_tensor(
                        out=ob, in0=po_, in1=bias_sb,
                        op=mybir.AluOpType.add)
                    nc.sync.dma_start(
                        out=out_d[tt * 128:(tt + 1) * 128, :], in_=ob)

    nc.compile()
    return nc


# ---------------------------------------------------------------------
# host side
# ---------------------------------------------------------------------

def _trig_tables(positions, n):
    """cos / signed-sin tables in transposed layout [128, n].

    Row p corresponds to head-dim d = p % 64; freq index d//2.  The sin
    table carries the rotate_half sign: -sin on even d, +sin on odd d,
    so that q' = q*cos + swap_pairs(q)*sin_signed.
    """
    inv = 1.0 / (ROPE_BASE ** (np.arange(HD // 2, dtype=np.float32)
                               / (HD // 2)))
    freqs = positions.astype(np.float32)[None, :] * inv[:, None]  # [32, n]
    cos = np.repeat(np.cos(freqs), 2, axis=0)  # [64, n]
    sin = np.repeat(np.sin(freqs), 2, axis=0)
    sign = np.where(np.arange(64) % 2 == 0, -1.0, 1.0).astype(np.float32)
    sin = sin * sign[:, None]
    return (np.ascontiguousarray(np.tile(cos, (2, 1))).astype(NP_BF16),
            np.ascontiguousarray(np.tile(sin, (2, 1))).astype(NP_BF16))


def _masks(qs):
    """0/1 mask tiles [128, 2*768]: scores^T layout [key-part, query-free].

    Packed per qbi block: [s0(128) s1(256) s4(256) s5(128)] matching the
    band-exact pes layout; subtiles 2,3 are fully in-band (no mask)."""
    p = np.arange(128)[:, None]
    m = np.zeros((128, 2, 768), np.float32)
    spec = [(0, 0, 128, 0), (1, 128, 256, 0),
            (4, 384, 256, 0), (5, 640, 128, 128)]
    for qi, qb in enumerate((0, QB)):
        for s, col, w, x0 in spec:
            x = x0 + np.arange(w)[None, :]
            pk = (qs - LOCAL_WINDOW) + qb + 128 * s + p  # global key pos
            band = (x >= 128 * s + p - LOCAL_WINDOW) & (x <= 128 * s + p)
            m[:, qi, col:col + w] = (band & (pk >= 0)).astype(np.float32)
    return np.ascontiguousarray(m.reshape(128, 2 * 768)).astype(NP_BF16)


def _host_inputs(x, Wqkv, Wproj, bproj):
    # 1/sqrt(hd) folded into the (linear) q projection
    Wq, Wk, Wv = Wqkv[0:C] * SCALE, Wqkv[C:2 * C], Wqkv[2 * C:3 * C]
    # slab layout: per o-tile [K(128) | Q(128)] interleaved, then V
    WqT, WkT, WvT = Wq.T, Wk.T, Wv.T
    slab = np.empty((C, 2 * C), np.float32)
    for ot in range(8):
        slab[:, ot * 256:ot * 256 + 128] = WkT[:, ot * 128:(ot + 1) * 128]
        slab[:, ot * 256 + 128:ot * 256 + 256] = \
            WqT[:, ot * 128:(ot + 1) * 128]
    wT = np.ascontiguousarray(
        np.concatenate([slab, WvT], axis=1)).astype(NP_BF16)
    wpT = np.ascontiguousarray(Wproj.T).astype(NP_BF16)
    biasb = np.ascontiguousarray(
        np.broadcast_to(bproj, (128, C))).astype(np.float32)

    in_maps = []
    for core in range(NCORES):
        b, qi = divmod(core, 4)
        qs = qi * QL
        qe = qs + QL
        lo = qe - KL
        xw = np.zeros((KL, C), np.float32)
        src_lo = max(lo, 0)
        xw[src_lo - lo:, :] = x[b, src_lo:qe, :]
        xT = np.ascontiguousarray(xw.T).astype(NP_BF16)

        cosq, sinq = _trig_tables(qs + np.arange(QL), QL)
        cosk, sink = _trig_tables(lo + np.arange(KL), KL)
        # validity column for [V|1]: 1.0 where the key row is a real
        # (non-padding) position, per v-time-tile, repeated per head
        jpos = lo + np.arange(KL)
        v8 = (jpos >= 0).astype(np.float32).reshape(8, 128)  # [tt, p]
        vone = np.ascontiguousarray(
            np.repeat(v8[:, :, None], H, axis=2)             # [tt, p, h]
            .transpose(1, 0, 2).reshape(128, 8 * H)).astype(NP_BF16)
        in_maps.append({
            "xT": xT, "wT": wT, "wpT": wpT, "biasb": biasb,
            "cosq": cosq, "sinq": sinq, "cosk": cosk, "sink": sink,
            "mask": _masks(qs), "vone": vone,
        })
    return in_maps


_NC_CACHE = {}


def _get_nc(reps=1):
    key = reps
    if key not in _NC_CACHE:
        _NC_CACHE[key] = build_nc(reps=reps)
    return _NC_CACHE[key]


def kernel(x, Wqkv, Wproj, bproj):
    x = np.asarray(x, dtype=np.float32)
    Wqkv = np.asarray(Wqkv, dtype=np.float32)
    Wproj = np.asarray(Wproj, dtype=np.float32)
    bproj = np.asarray(bproj, dtype=np.float32)
    nc = _get_nc()
    in_maps = _host_inputs(x, Wqkv, Wproj, bproj)
    res = run_bass_kernel_spmd(nc, in_maps, list(range(NCORES)))
    out = np.empty((B, T, C), np.float32)
    for core in range(NCORES):
        b, qi = divmod(core, 4)
        out[b, qi * QL:(qi + 1) * QL, :] = res.results[core]["out"]
    return out
